# revision 12
# baseline (speedup 1.0000x reference)
"""Trainium2 Bass kernel for DDN depth-focal loss (nn_DDNLoss) — v2.

Data-parallel over batch B=8 across 8 NeuronCores (1 image per core).
Each core computes sum_pixels(weight * focal(depth_logits, target)); host
sums the 8 partials and divides by B*H*W.

v2 design (vs v1 baseline at 193us):
  - Rasterization: ONE fp32 matmul. Box ranked r (far->near) contributes
    rowmask*2^r (x) colmask; PSUM sums distinct powers of two exactly, so
    the per-pixel value v has the NEAREST covering box's rank as its top
    bit. No PSUM max-combining, no depth-quantization keys.
  - Winner decode: t = 2^r* by clearing the mantissa (bitcast + AND).
  - Gather: 33 fused scalar_tensor_tensor ops, (t == 2^(s-1)) * slot_s
    into an s-major scratch, then a bf16 pairwise tree-sum (one nonzero
    term per pixel -> exact). No separate mask materialization.
  - Softmax sum: fp8 logits streamed, ACT exp -> bf16, two bf16 2x
    tensor_tensor folds (88 = 2*2*22 channel layout) + one tensor_reduce.
  - lse via exponent-field bit trick on DVE (avoids a Ln table load).
  - Focal epilogue in bf16; gpsimd partition_all_reduce for the scalar.
"""

import numpy as np
import ml_dtypes

import concourse.bacc as bacc
import concourse.bass as bass
import concourse.mybir as mybir
from concourse import bass_isa, tile
from concourse.bass_utils import run_bass_kernel_spmd

# Problem constants (hardcoded per harness contract).
B, C, H, W, N = 8, 81, 96, 320, 32
P = 128
HW = H * W              # 30720
J = HW // P             # 240 pixel columns per partition
CPS = 88                # sum-region channels padded (81 -> 88 = 2*2*22)
CH = CPS // 2           # 44
CQ = CPS // 4           # 22
NSLOT = 33              # bg + 32 rank slots
STRIPJ = [60, 60, 60, 60]
NSTRIP = len(STRIPJ)
JOFF = [sum(STRIPJ[:i]) for i in range(NSTRIP + 1)]

ALPHA = 0.25
FG_W, BG_W = 13.0, 1.0
DEPTH_MIN, DEPTH_MAX, NUM_BINS = 0.001, 60.0, 80
BIN_SIZE = 2.0 * (DEPTH_MAX - DEPTH_MIN) / (NUM_BINS * (1 + NUM_BINS))
PAD_LOGIT = -20.0
LN2 = float(np.log(2.0))

F32 = mybir.dt.float32
BF16 = mybir.dt.bfloat16
FP8 = mybir.dt.float8e4
I32 = mybir.dt.int32
U8 = mybir.dt.uint8
Alu = mybir.AluOpType
Act = mybir.ActivationFunctionType

_CACHE = {}
LAST_RESULT = [None]


def _build():
    nc = bacc.Bacc("TRN2", target_bir_lowering=False, debug=False)

    xsum = nc.dram_tensor("xsum", [P, J * CPS], FP8, kind="ExternalInput")
    xg = nc.dram_tensor("xg", [P, NSLOT * J], BF16, kind="ExternalInput")
    SM = 4 + 4 + 1 + W + H
    smalls = nc.dram_tensor("smalls", [N, SM], F32, kind="ExternalInput")
    outv = nc.dram_tensor("outv", [1, 1], F32, kind="ExternalOutput")

    HFS = J * CH            # 10560 elements per half
    SFS_ = [js * CH for js in STRIPJ]
    SOFF = [jo * CPS for jo in JOFF]    # element offset of strip s in xsum
    SFSMAX = max(SFS_)

    with tile.TileContext(nc) as tc:
        with (
            tc.tile_pool(name="xs", bufs=1) as xsp,
            tc.tile_pool(name="es", bufs=2) as esp,
            tc.tile_pool(name="fold", bufs=2) as fp_,
            tc.tile_pool(name="map", bufs=1) as mapp,
            tc.tile_pool(name="sml", bufs=1) as smlp,
            tc.tile_pool(name="drp", bufs=1, space="DRAM") as drp,
            tc.tile_pool(name="ps", bufs=2, space="PSUM") as psp,
        ):
            # ---- ACT table warmup: load exp set while DMAs stream ----
            warm = smlp.tile([P, 1], F32)
            nc.vector.memset(warm[:], 0.0)
            nc.scalar.activation(warm[:], warm[:], Act.Exp)

            # ---- input DMAs ----
            # strip 0 goes first on the scalar ring so exp0 starts ASAP;
            # strips 1-3 stream on the sync ring in parallel
            xst = xsp.tile([P, J * CPS], FP8)
            nc.scalar.dma_start(xst[:, SOFF[0] : SOFF[1]], xsum[:, SOFF[0] : SOFF[1]])
            sm_t = smlp.tile([N, SM], F32)
            nc.scalar.dma_start(sm_t[:], smalls[:])
            box_t = sm_t[:, 0:4]
            sgn_t = sm_t[:, 4:8]
            pw2_t = sm_t[:, 8:9]
            iotw_t = sm_t[:, 9 : 9 + W]
            ioth_t = sm_t[:, 9 + W : 9 + W + H]

            # remaining xsum strips on the sync ring
            for s in range(1, NSTRIP):
                nc.sync.dma_start(
                    xst[:, SOFF[s] : SOFF[s + 1]],
                    xsum[:, SOFF[s] : SOFF[s + 1]],
                )
            # gather planes ride the Activation DGE ring (issued at t~0)
            xgt = xsp.tile([P, NSLOT * J], BF16)
            nc.scalar.dma_start(xgt[:], xg[:])

            # ---- floor(u1,v1)/ceil(u2,v2): convert then fix up ----
            bxi = smlp.tile([N, 4], I32)
            nc.vector.tensor_copy(bxi[:], box_t)
            bxf = smlp.tile([N, 4], F32)
            nc.vector.tensor_copy(bxf[:], bxi[:])
            dlt = smlp.tile([N, 4], F32)
            nc.vector.tensor_tensor(dlt[:, 0:2], bxf[:, 0:2], sm_t[:, 0:2], Alu.is_gt)
            nc.vector.tensor_tensor(dlt[:, 2:4], bxf[:, 2:4], sm_t[:, 2:4], Alu.is_lt)
            nc.vector.tensor_tensor(dlt[:], dlt[:], sgn_t, Alu.mult)
            nc.vector.tensor_tensor(bxf[:], bxf[:], dlt[:], Alu.add)

            # ---- interval masks ----
            mwa = smlp.tile([N, W], F32)
            nc.vector.tensor_scalar(mwa[:], iotw_t, bxf[:, 0:1], None, Alu.is_ge)
            mw = smlp.tile([N, W], F32)
            nc.vector.tensor_scalar(mw[:], iotw_t, bxf[:, 2:3], None, Alu.is_lt)
            nc.vector.tensor_tensor(mw[:], mw[:], mwa[:], Alu.mult)

            mha = smlp.tile([N, H], F32)
            nc.vector.tensor_scalar(mha[:], ioth_t, bxf[:, 1:2], None, Alu.is_ge)
            mhb = smlp.tile([N, H], F32)
            nc.vector.tensor_scalar(mhb[:], ioth_t, bxf[:, 3:4], None, Alu.is_lt)
            mhs = smlp.tile([N, H], F32)
            nc.vector.scalar_tensor_tensor(
                mhs[:], mha[:], pw2_t, mhb[:], Alu.mult, Alu.mult
            )

            # ---- raster: one fp32 matmul, v = sum of 2^rank over boxes ----
            ps = psp.tile([H, W], F32, tag="ps")
            nc.tensor.matmul(ps[:], mhs[:], mw[:], start=True, stop=True)
            # drain PSUM -> SBUF on DVE, bounce DMAs issued from DVE so the
            # reshape completes while strip 0 is still streaming/exping
            vhw = smlp.tile([H, W], F32)
            nc.scalar.activation(vhw[:], ps[:], Act.Identity)
            zb = drp.tile([1, HW], F32)
            nc.scalar.dma_start(zb[:], vhw[:])

            sred = mapp.tile([P, J], F32)
            scr = mapp.tile([P, NSLOT * J], BF16)
            vmap = mapp.tile([P, J], F32)
            ti = mapp.tile([P, J], I32)
            tb = mapp.tile([P, J], BF16)
            wt0 = mapp.tile([P, J], BF16)

            def strip_folds(s):
                sfs = SFS_[s]
                est = esp.tile([P, 2 * SFSMAX], BF16, tag="est")
                nc.scalar.activation(
                    est[:, 0 : 2 * sfs].rearrange("p (h f) -> p h f", h=2),
                    xst[:, SOFF[s] : SOFF[s + 1]].rearrange(
                        "p (h f) -> p h f", h=2
                    ),
                    Act.Exp,
                )
                fA = fp_.tile([P, SFSMAX], BF16, tag="fA")
                fA_w = fA[:, 0:sfs].rearrange(
                    "p (c2 j c) -> p j c2 c", c2=2, c=CQ
                )
                a4 = est[:, 0:sfs].rearrange("p (j c2 c) -> p j c2 c", c2=2, c=CQ)
                b4 = est[:, sfs : 2 * sfs].rearrange(
                    "p (j c2 c) -> p j c2 c", c2=2, c=CQ
                )
                nc.vector.tensor_tensor(fA_w, a4, b4, Alu.add)
                fB = fp_.tile([P, SFSMAX // 2], BF16, tag="fB")
                nc.vector.tensor_tensor(
                    fB[:, 0 : sfs // 2],
                    fA[:, 0 : sfs // 2],
                    fA[:, sfs // 2 : sfs],
                    Alu.add,
                )
                nc.vector.tensor_reduce(
                    sred[:, JOFF[s] : JOFF[s + 1]],
                    fB[:, 0 : sfs // 2].rearrange("p (j c) -> p j c", c=CQ),
                    axis=mybir.AxisListType.X,
                    op=Alu.add,
                )

            def gather_op(g):
                if g == 0:
                    # background slot: t < 1  <=>  uncovered
                    nc.vector.scalar_tensor_tensor(
                        scr[:, 0:J], tb[:], 1.0, xgt[:, 0:J], Alu.is_lt, Alu.mult
                    )
                else:
                    nc.vector.scalar_tensor_tensor(
                        scr[:, g * J : (g + 1) * J],
                        tb[:],
                        float(2.0 ** (g - 1)),
                        xgt[:, g * J : (g + 1) * J],
                        Alu.is_equal,
                        Alu.mult,
                    )

            # strip 0 folds overlap the vmap reshape (SBUF->SBUF: the
            # pixel-linear iteration orders of (96,320) and (128,240) match)
            strip_folds(0)
            nc.scalar.dma_start(vmap[:], zb[:])

            # winner decode: t = 2^r* (clear mantissa), bf16 copy
            with tc.high_priority():
                nc.vector.tensor_scalar(
                    ti[:], vmap[:].bitcast(I32), 0x7F800000, None, Alu.bitwise_and
                )
                nc.vector.tensor_copy(tb[:], ti[:].bitcast(F32))
                # weights: wt0 = 12 * covered (bf16; 12/13 exact)
                nc.vector.tensor_scalar(
                    wt0[:], vmap[:], 1.0, FG_W - BG_W, Alu.is_ge, Alu.mult
                )

            # remaining strips with gather ops interleaved; first tree
            # halves fire as soon as their slots are complete
            t8a = mapp.tile([P, 8 * J], BF16)
            t8b = mapp.tile([P, 8 * J], BF16)
            gi = 0
            batches = [10, 11, 12]
            for s in range(1, NSTRIP):
                strip_folds(s)
                for g in range(gi, min(gi + batches[s - 1], NSLOT)):
                    gather_op(g)
                gi = min(gi + batches[s - 1], NSLOT)
                if gi >= 16 and gi - batches[s - 1] < 16:
                    nc.vector.tensor_tensor(
                        t8a[:], scr[:, 0 : 8 * J], scr[:, 8 * J : 16 * J], Alu.add
                    )
            for g in range(gi, NSLOT):
                gather_op(g)
            nc.vector.tensor_tensor(
                t8b[:], scr[:, 16 * J : 24 * J], scr[:, 24 * J : 32 * J], Alu.add
            )

            # ---- gather tree tail: t8a+t8b -> 4/2/1, + slot 32 ----
            t8 = mapp.tile([P, 8 * J], BF16)
            nc.vector.tensor_tensor(t8[:], t8a[:], t8b[:], Alu.add)
            t4 = mapp.tile([P, 4 * J], BF16)
            nc.vector.tensor_tensor(
                t4[:], t8[:, 0 : 4 * J], t8[:, 4 * J : 8 * J], Alu.add
            )
            t2 = mapp.tile([P, 2 * J], BF16)
            nc.vector.tensor_tensor(
                t2[:], t4[:, 0 : 2 * J], t4[:, 2 * J : 4 * J], Alu.add
            )
            t1 = mapp.tile([P, J], BF16)
            nc.vector.tensor_tensor(t1[:], t2[:, 0:J], t2[:, J : 2 * J], Alu.add)
            gat = mapp.tile([P, J], BF16)
            nc.vector.tensor_tensor(
                gat[:], t1[:], scr[:, 32 * J : 33 * J], Alu.add
            )

            # ---- lse = ln(S) via exponent bit trick (bf16 out) ----
            # ln(S) ~ ln2 * (bits(S)/2^23 - 127 + sigma); sigma tuned so the
            # piecewise-linear log2 has ~zero mean error.
            SIG = 0.0573
            lse = mapp.tile([P, J], BF16)
            nc.vector.tensor_scalar(
                lse[:],
                sred[:].bitcast(I32),
                LN2 / (1 << 23),
                -(127.0 - SIG) * LN2,
                Alu.mult,
                Alu.add,
            )

            # ---- focal epilogue (bf16) ----
            logp = mapp.tile([P, J], BF16)
            nc.vector.tensor_tensor(logp[:], gat[:], lse[:], Alu.subtract)
            pt = mapp.tile([P, J], BF16)
            nc.scalar.activation(pt[:], logp[:], Act.Exp)
            um = mapp.tile([P, J], BF16)
            nc.scalar.activation(um[:], pt[:], Act.Identity, scale=-1.0, bias=1.0)
            tmp = mapp.tile([P, J], BF16)
            nc.vector.scalar_tensor_tensor(
                tmp[:], um[:], -ALPHA, um[:], Alu.mult, Alu.mult
            )
            wl = mapp.tile([P, J], BF16)
            nc.vector.scalar_tensor_tensor(
                wl[:], wt0[:], 1.0, logp[:], Alu.add, Alu.mult
            )
            junk = mapp.tile([P, J], BF16)
            nc.vector.tensor_tensor(junk[:], tmp[:], wl[:], Alu.mult)
            acc = mapp.tile([P, 1], F32)
            nc.vector.tensor_reduce(
                acc[:], junk[:], axis=mybir.AxisListType.X, op=Alu.add
            )
            tot = mapp.tile([P, 1], F32)
            nc.gpsimd.partition_all_reduce(
                tot[:], acc[:], channels=P, reduce_op=bass_isa.ReduceOp.add
            )
            nc.sync.dma_start(outv[:], tot[0:1, 0:1])

    nc.finalize()
    return nc


def _ref_bin(d):
    """Per-box target bin, replicating the reference's float32 LID binning."""
    d = np.float32(d)
    a = np.float32(1.0) + np.float32(8.0) * (d - np.float32(DEPTH_MIN)) / np.float32(
        BIN_SIZE
    )
    idx = np.float32(-0.5) + np.float32(0.5) * np.sqrt(a, dtype=np.float32)
    return int(np.int32(idx))


def _host_prep(depth_logits, gt_boxes2d, gt_center_depth):
    xt = np.transpose(depth_logits, (0, 2, 3, 1)).reshape(B, HW, C)
    boxes = gt_boxes2d.reshape(B, N, 4)
    depths = gt_center_depth.reshape(B, N)

    fbox = np.concatenate(
        [np.floor(boxes[:, :, :2]), np.ceil(boxes[:, :, 2:])], axis=2
    )

    SM = 4 + 4 + 1 + W + H
    xsum = np.empty((B, P, J * CPS), ml_dtypes.float8_e4m3fn)
    xg = np.empty((B, P, NSLOT * J), ml_dtypes.bfloat16)
    smalls = np.empty((B, N, SM), np.float32)

    for b in range(B):
        # rank: farthest depth = rank 0, nearest = rank N-1
        order = np.argsort(-depths[b], kind="stable")
        smalls[b, :, 0:4] = boxes[b][order]
        smalls[b, :, 4:8] = np.array([-1.0, -1.0, 1.0, 1.0], np.float32)
        smalls[b, :, 8] = (2.0 ** np.arange(N)).astype(np.float32)
        smalls[b, :, 9 : 9 + W] = np.arange(W, dtype=np.float32)
        smalls[b, :, 9 + W : 9 + W + H] = np.arange(H, dtype=np.float32)

        # f32-exactness guard for the power-sum raster: counts per pixel
        u1 = fbox[b, :, 0].astype(int).clip(0, W)
        v1 = fbox[b, :, 1].astype(int).clip(0, H)
        u2 = fbox[b, :, 2].astype(int).clip(0, W)
        v2 = fbox[b, :, 3].astype(int).clip(0, H)
        cnt = np.zeros((H, W), np.int32)
        for n in range(N):
            cnt[v1[n] : v2[n], u1[n] : u2[n]] += 1
        assert cnt.max() <= 23, "overlap too deep for exact f32 power-sum"

        # sum region: 88 channels, strip-major [stripS: halfA | halfB]
        xb = np.full((HW, CPS), PAD_LOGIT, np.float32)
        xb[:, :C] = xt[b]
        xb = xb.reshape(P, J, CPS)
        blocks = []
        for s in range(NSTRIP):
            seg = xb[:, JOFF[s] : JOFF[s + 1], :]
            blocks.append(seg[:, :, :CH].reshape(P, STRIPJ[s] * CH))
            blocks.append(seg[:, :, CH:].reshape(P, STRIPJ[s] * CH))
        xsum[b] = np.concatenate(blocks, axis=1).astype(
            ml_dtypes.float8_e4m3fn
        )

        # gather region: slot-major planes; slot 0 = bg, slot s = rank s-1
        chans = [NUM_BINS] + [_ref_bin(depths[b, n]) for n in order]
        g = np.empty((NSLOT, P, J), np.float32)
        xpix = xt[b].reshape(P, J, C)
        for s, ch in enumerate(chans):
            g[s] = xpix[:, :, ch]
        xg[b] = np.transpose(g, (1, 0, 2)).reshape(P, NSLOT * J).astype(
            ml_dtypes.bfloat16
        )

    return xsum, xg, smalls


def kernel(depth_logits, gt_boxes2d, gt_boxes3d, gt_center_depth, num_gt_per_img):
    depth_logits = np.asarray(depth_logits, dtype=np.float32)
    gt_boxes2d = np.asarray(gt_boxes2d, dtype=np.float32)
    gt_center_depth = np.asarray(gt_center_depth, dtype=np.float32)
    assert int(num_gt_per_img) == N

    xsum, xg, smalls = _host_prep(depth_logits, gt_boxes2d, gt_center_depth)

    if "nc" not in _CACHE:
        _CACHE["nc"] = _build()
    nc = _CACHE["nc"]

    in_maps = []
    for b in range(B):
        in_maps.append(
            {
                "xsum": np.ascontiguousarray(xsum[b]),
                "xg": np.ascontiguousarray(xg[b]),
                "smalls": np.ascontiguousarray(smalls[b]),
            }
        )

    res = run_bass_kernel_spmd(nc, in_maps, core_ids=list(range(B)))
    LAST_RESULT[0] = res
    total = 0.0
    for b in range(B):
        total += float(res.results[b]["outv"][0, 0])
    return np.float32(total / (B * H * W))


# revision 13
# speedup vs baseline: 1.0683x; 1.0683x over previous
"""Trainium2 Bass kernel for DDN depth-focal loss (nn_DDNLoss) — v2.

Data-parallel over batch B=8 across 8 NeuronCores (1 image per core).
Each core computes sum_pixels(weight * focal(depth_logits, target)); host
sums the 8 partials and divides by B*H*W.

v2 design (vs v1 baseline at 193us):
  - Rasterization: ONE fp32 matmul. Box ranked r (far->near) contributes
    rowmask*2^r (x) colmask; PSUM sums distinct powers of two exactly, so
    the per-pixel value v has the NEAREST covering box's rank as its top
    bit. No PSUM max-combining, no depth-quantization keys.
  - Winner decode: t = 2^r* by clearing the mantissa (bitcast + AND).
  - Gather: 33 fused scalar_tensor_tensor ops, (t == 2^(s-1)) * slot_s
    into an s-major scratch, then a bf16 pairwise tree-sum (one nonzero
    term per pixel -> exact). No separate mask materialization.
  - Softmax sum: fp8 logits streamed, ACT exp -> bf16, two bf16 2x
    tensor_tensor folds (88 = 2*2*22 channel layout) + one tensor_reduce.
  - lse via exponent-field bit trick on DVE (avoids a Ln table load).
  - Focal epilogue in bf16; gpsimd partition_all_reduce for the scalar.
"""

import numpy as np
import ml_dtypes

import concourse.bacc as bacc
import concourse.bass as bass
import concourse.mybir as mybir
from concourse import bass_isa, tile
from concourse.bass_utils import run_bass_kernel_spmd

# Problem constants (hardcoded per harness contract).
B, C, H, W, N = 8, 81, 96, 320, 32
P = 128
HW = H * W              # 30720
J = HW // P             # 240 pixel columns per partition
CPS = 88                # sum-region channels padded (81 -> 88 = 2*2*22)
CH = CPS // 2           # 44
CQ = CPS // 4           # 22
NSLOT = 33              # bg + 32 rank slots
STRIPJ = [60, 60, 60, 60]
NSTRIP = len(STRIPJ)
JOFF = [sum(STRIPJ[:i]) for i in range(NSTRIP + 1)]

ALPHA = 0.25
FG_W, BG_W = 13.0, 1.0
DEPTH_MIN, DEPTH_MAX, NUM_BINS = 0.001, 60.0, 80
BIN_SIZE = 2.0 * (DEPTH_MAX - DEPTH_MIN) / (NUM_BINS * (1 + NUM_BINS))
PAD_LOGIT = -20.0
LN2 = float(np.log(2.0))

F32 = mybir.dt.float32
BF16 = mybir.dt.bfloat16
FP8 = mybir.dt.float8e4
I32 = mybir.dt.int32
U8 = mybir.dt.uint8
Alu = mybir.AluOpType
Act = mybir.ActivationFunctionType

_CACHE = {}
LAST_RESULT = [None]


def _build():
    nc = bacc.Bacc("TRN2", target_bir_lowering=False, debug=False)

    xsum = nc.dram_tensor("xsum", [P, J * CPS], FP8, kind="ExternalInput")
    xg = nc.dram_tensor("xg", [P, NSLOT * J], BF16, kind="ExternalInput")
    SM = 4 + 4 + 1 + W + H
    smalls = nc.dram_tensor("smalls", [N, SM], F32, kind="ExternalInput")
    outv = nc.dram_tensor("outv", [1, 1], F32, kind="ExternalOutput")

    HFS = J * CH            # 10560 elements per half
    SFS_ = [js * CH for js in STRIPJ]
    SOFF = [jo * CPS for jo in JOFF]    # element offset of strip s in xsum
    SFSMAX = max(SFS_)

    with tile.TileContext(nc) as tc:
        with (
            tc.tile_pool(name="xs", bufs=1) as xsp,
            tc.tile_pool(name="es", bufs=2) as esp,
            tc.tile_pool(name="fold", bufs=2) as fp_,
            tc.tile_pool(name="map", bufs=1) as mapp,
            tc.tile_pool(name="sml", bufs=1) as smlp,
            tc.tile_pool(name="drp", bufs=1, space="DRAM") as drp,
            tc.tile_pool(name="ps", bufs=2, space="PSUM") as psp,
        ):
            # ---- ACT table warmup: load exp set while DMAs stream ----
            warm = smlp.tile([P, 1], F32)
            nc.vector.memset(warm[:], 0.0)
            nc.scalar.activation(warm[:], warm[:], Act.Exp)

            # ---- input DMAs ----
            # strip 0 goes first on the scalar ring so exp0 starts ASAP;
            # strips 1-3 stream on the sync ring in parallel
            xst = xsp.tile([P, J * CPS], FP8)
            nc.scalar.dma_start(xst[:, SOFF[0] : SOFF[1]], xsum[:, SOFF[0] : SOFF[1]])
            sm_t = smlp.tile([N, SM], F32)
            nc.scalar.dma_start(sm_t[:], smalls[:])
            box_t = sm_t[:, 0:4]
            sgn_t = sm_t[:, 4:8]
            pw2_t = sm_t[:, 8:9]
            iotw_t = sm_t[:, 9 : 9 + W]
            ioth_t = sm_t[:, 9 + W : 9 + W + H]

            # remaining xsum strips on the sync ring
            for s in range(1, NSTRIP):
                nc.sync.dma_start(
                    xst[:, SOFF[s] : SOFF[s + 1]],
                    xsum[:, SOFF[s] : SOFF[s + 1]],
                )
            # gather planes ride the Activation DGE ring (issued at t~0)
            xgt = xsp.tile([P, NSLOT * J], BF16)
            nc.scalar.dma_start(xgt[:], xg[:])

            # ---- floor(u1,v1)/ceil(u2,v2): convert then fix up ----
            bxi = smlp.tile([N, 4], I32)
            nc.vector.tensor_copy(bxi[:], box_t)
            bxf = smlp.tile([N, 4], F32)
            nc.vector.tensor_copy(bxf[:], bxi[:])
            dlt = smlp.tile([N, 4], F32)
            nc.vector.tensor_tensor(dlt[:, 0:2], bxf[:, 0:2], sm_t[:, 0:2], Alu.is_gt)
            nc.vector.tensor_tensor(dlt[:, 2:4], bxf[:, 2:4], sm_t[:, 2:4], Alu.is_lt)
            nc.vector.tensor_tensor(dlt[:], dlt[:], sgn_t, Alu.mult)
            nc.vector.tensor_tensor(bxf[:], bxf[:], dlt[:], Alu.add)

            # ---- interval masks ----
            mwa = smlp.tile([N, W], F32)
            nc.vector.tensor_scalar(mwa[:], iotw_t, bxf[:, 0:1], None, Alu.is_ge)
            mw = smlp.tile([N, W], F32)
            nc.vector.tensor_scalar(mw[:], iotw_t, bxf[:, 2:3], None, Alu.is_lt)
            nc.vector.tensor_tensor(mw[:], mw[:], mwa[:], Alu.mult)

            mha = smlp.tile([N, H], F32)
            nc.vector.tensor_scalar(mha[:], ioth_t, bxf[:, 1:2], None, Alu.is_ge)
            mhb = smlp.tile([N, H], F32)
            nc.vector.tensor_scalar(mhb[:], ioth_t, bxf[:, 3:4], None, Alu.is_lt)
            mhs = smlp.tile([N, H], F32)
            nc.vector.scalar_tensor_tensor(
                mhs[:], mha[:], pw2_t, mhb[:], Alu.mult, Alu.mult
            )

            # ---- raster: one fp32 matmul, v = sum of 2^rank over boxes ----
            ps = psp.tile([H, W], F32, tag="ps")
            nc.tensor.matmul(ps[:], mhs[:], mw[:], start=True, stop=True)
            # drain PSUM -> SBUF on DVE, bounce DMAs issued from DVE so the
            # reshape completes while strip 0 is still streaming/exping
            vhw = smlp.tile([H, W], F32)
            nc.vector.tensor_copy(vhw[:], ps[:])
            zb = drp.tile([1, HW], F32)
            nc.scalar.dma_start(zb[:], vhw[:])

            sred = mapp.tile([P, J], F32)
            scr = mapp.tile([P, NSLOT * J], BF16)
            vmap = mapp.tile([P, J], F32)
            ti = mapp.tile([P, J], I32)
            tb = mapp.tile([P, J], BF16)
            wt0 = mapp.tile([P, J], BF16)

            def strip_folds(s):
                sfs = SFS_[s]
                est = esp.tile([P, 2 * SFSMAX], BF16, tag="est")
                nc.scalar.activation(
                    est[:, 0 : 2 * sfs].rearrange("p (h f) -> p h f", h=2),
                    xst[:, SOFF[s] : SOFF[s + 1]].rearrange(
                        "p (h f) -> p h f", h=2
                    ),
                    Act.Exp,
                )
                fA = fp_.tile([P, SFSMAX], BF16, tag="fA")
                fA_w = fA[:, 0:sfs].rearrange(
                    "p (c2 j c) -> p j c2 c", c2=2, c=CQ
                )
                a4 = est[:, 0:sfs].rearrange("p (j c2 c) -> p j c2 c", c2=2, c=CQ)
                b4 = est[:, sfs : 2 * sfs].rearrange(
                    "p (j c2 c) -> p j c2 c", c2=2, c=CQ
                )
                nc.vector.tensor_tensor(fA_w, a4, b4, Alu.add)
                fB = fp_.tile([P, SFSMAX // 2], BF16, tag="fB")
                nc.vector.tensor_tensor(
                    fB[:, 0 : sfs // 2],
                    fA[:, 0 : sfs // 2],
                    fA[:, sfs // 2 : sfs],
                    Alu.add,
                )
                nc.vector.tensor_reduce(
                    sred[:, JOFF[s] : JOFF[s + 1]],
                    fB[:, 0 : sfs // 2].rearrange("p (j c) -> p j c", c=CQ),
                    axis=mybir.AxisListType.X,
                    op=Alu.add,
                )

            def gather_op(g):
                if g == 0:
                    # background slot: t < 1  <=>  uncovered
                    nc.vector.scalar_tensor_tensor(
                        scr[:, 0:J], tb[:], 1.0, xgt[:, 0:J], Alu.is_lt, Alu.mult
                    )
                else:
                    nc.vector.scalar_tensor_tensor(
                        scr[:, g * J : (g + 1) * J],
                        tb[:],
                        float(2.0 ** (g - 1)),
                        xgt[:, g * J : (g + 1) * J],
                        Alu.is_equal,
                        Alu.mult,
                    )

            # strip 0 folds overlap the vmap reshape (SBUF->SBUF: the
            # pixel-linear iteration orders of (96,320) and (128,240) match)
            strip_folds(0)
            nc.scalar.dma_start(vmap[:], zb[:])

            # winner decode: t = 2^r* (clear mantissa), bf16 copy
            with tc.high_priority():
                nc.vector.tensor_scalar(
                    ti[:], vmap[:].bitcast(I32), 0x7F800000, None, Alu.bitwise_and
                )
                nc.vector.tensor_copy(tb[:], ti[:].bitcast(F32))
                # weights: wt0 = 12 * covered (bf16; 12/13 exact)
                nc.vector.tensor_scalar(
                    wt0[:], vmap[:], 1.0, FG_W - BG_W, Alu.is_ge, Alu.mult
                )

            # remaining strips with gather ops interleaved; first tree
            # halves fire as soon as their slots are complete
            t8a = mapp.tile([P, 8 * J], BF16)
            t8b = mapp.tile([P, 8 * J], BF16)
            gi = 0
            batches = [10, 11, 12]
            for s in range(1, NSTRIP):
                strip_folds(s)
                for g in range(gi, min(gi + batches[s - 1], NSLOT)):
                    gather_op(g)
                gi = min(gi + batches[s - 1], NSLOT)
                if gi >= 16 and gi - batches[s - 1] < 16:
                    nc.vector.tensor_tensor(
                        t8a[:], scr[:, 0 : 8 * J], scr[:, 8 * J : 16 * J], Alu.add
                    )
            for g in range(gi, NSLOT):
                gather_op(g)
            nc.vector.tensor_tensor(
                t8b[:], scr[:, 16 * J : 24 * J], scr[:, 24 * J : 32 * J], Alu.add
            )

            # ---- gather tree tail: t8a+t8b -> 4/2/1, + slot 32 ----
            t8 = mapp.tile([P, 8 * J], BF16)
            nc.vector.tensor_tensor(t8[:], t8a[:], t8b[:], Alu.add)
            t4 = mapp.tile([P, 4 * J], BF16)
            nc.vector.tensor_tensor(
                t4[:], t8[:, 0 : 4 * J], t8[:, 4 * J : 8 * J], Alu.add
            )
            t2 = mapp.tile([P, 2 * J], BF16)
            nc.vector.tensor_tensor(
                t2[:], t4[:, 0 : 2 * J], t4[:, 2 * J : 4 * J], Alu.add
            )
            t1 = mapp.tile([P, J], BF16)
            nc.vector.tensor_tensor(t1[:], t2[:, 0:J], t2[:, J : 2 * J], Alu.add)
            gat = mapp.tile([P, J], BF16)
            nc.vector.tensor_tensor(
                gat[:], t1[:], scr[:, 32 * J : 33 * J], Alu.add
            )

            # ---- lse = ln(S) via exponent bit trick (bf16 out) ----
            # ln(S) ~ ln2 * (bits(S)/2^23 - 127 + sigma); sigma tuned so the
            # piecewise-linear log2 has ~zero mean error.
            SIG = 0.0573
            lse = mapp.tile([P, J], BF16)
            nc.vector.tensor_scalar(
                lse[:],
                sred[:].bitcast(I32),
                LN2 / (1 << 23),
                -(127.0 - SIG) * LN2,
                Alu.mult,
                Alu.add,
            )

            # ---- focal epilogue (bf16) ----
            logp = mapp.tile([P, J], BF16)
            nc.vector.tensor_tensor(logp[:], gat[:], lse[:], Alu.subtract)
            pt = mapp.tile([P, J], BF16)
            nc.scalar.activation(pt[:], logp[:], Act.Exp)
            um = mapp.tile([P, J], BF16)
            nc.scalar.activation(um[:], pt[:], Act.Identity, scale=-1.0, bias=1.0)
            tmp = mapp.tile([P, J], BF16)
            nc.vector.scalar_tensor_tensor(
                tmp[:], um[:], -ALPHA, um[:], Alu.mult, Alu.mult
            )
            wl = mapp.tile([P, J], BF16)
            nc.vector.scalar_tensor_tensor(
                wl[:], wt0[:], 1.0, logp[:], Alu.add, Alu.mult
            )
            junk = mapp.tile([P, J], BF16)
            nc.vector.tensor_tensor(junk[:], tmp[:], wl[:], Alu.mult)
            acc = mapp.tile([P, 1], F32)
            nc.vector.tensor_reduce(
                acc[:], junk[:], axis=mybir.AxisListType.X, op=Alu.add
            )
            tot = mapp.tile([P, 1], F32)
            nc.gpsimd.partition_all_reduce(
                tot[:], acc[:], channels=P, reduce_op=bass_isa.ReduceOp.add
            )
            nc.sync.dma_start(outv[:], tot[0:1, 0:1])

    nc.finalize()
    return nc


def _ref_bin(d):
    """Per-box target bin, replicating the reference's float32 LID binning."""
    d = np.float32(d)
    a = np.float32(1.0) + np.float32(8.0) * (d - np.float32(DEPTH_MIN)) / np.float32(
        BIN_SIZE
    )
    idx = np.float32(-0.5) + np.float32(0.5) * np.sqrt(a, dtype=np.float32)
    return int(np.int32(idx))


def _host_prep(depth_logits, gt_boxes2d, gt_center_depth):
    xt = np.transpose(depth_logits, (0, 2, 3, 1)).reshape(B, HW, C)
    boxes = gt_boxes2d.reshape(B, N, 4)
    depths = gt_center_depth.reshape(B, N)

    fbox = np.concatenate(
        [np.floor(boxes[:, :, :2]), np.ceil(boxes[:, :, 2:])], axis=2
    )

    SM = 4 + 4 + 1 + W + H
    xsum = np.empty((B, P, J * CPS), ml_dtypes.float8_e4m3fn)
    xg = np.empty((B, P, NSLOT * J), ml_dtypes.bfloat16)
    smalls = np.empty((B, N, SM), np.float32)

    for b in range(B):
        # rank: farthest depth = rank 0, nearest = rank N-1
        order = np.argsort(-depths[b], kind="stable")
        smalls[b, :, 0:4] = boxes[b][order]
        smalls[b, :, 4:8] = np.array([-1.0, -1.0, 1.0, 1.0], np.float32)
        smalls[b, :, 8] = (2.0 ** np.arange(N)).astype(np.float32)
        smalls[b, :, 9 : 9 + W] = np.arange(W, dtype=np.float32)
        smalls[b, :, 9 + W : 9 + W + H] = np.arange(H, dtype=np.float32)

        # f32-exactness guard for the power-sum raster: counts per pixel
        u1 = fbox[b, :, 0].astype(int).clip(0, W)
        v1 = fbox[b, :, 1].astype(int).clip(0, H)
        u2 = fbox[b, :, 2].astype(int).clip(0, W)
        v2 = fbox[b, :, 3].astype(int).clip(0, H)
        cnt = np.zeros((H, W), np.int32)
        for n in range(N):
            cnt[v1[n] : v2[n], u1[n] : u2[n]] += 1
        assert cnt.max() <= 23, "overlap too deep for exact f32 power-sum"

        # sum region: 88 channels, strip-major [stripS: halfA | halfB]
        xb = np.full((HW, CPS), PAD_LOGIT, np.float32)
        xb[:, :C] = xt[b]
        xb = xb.reshape(P, J, CPS)
        blocks = []
        for s in range(NSTRIP):
            seg = xb[:, JOFF[s] : JOFF[s + 1], :]
            blocks.append(seg[:, :, :CH].reshape(P, STRIPJ[s] * CH))
            blocks.append(seg[:, :, CH:].reshape(P, STRIPJ[s] * CH))
        xsum[b] = np.concatenate(blocks, axis=1).astype(
            ml_dtypes.float8_e4m3fn
        )

        # gather region: slot-major planes; slot 0 = bg, slot s = rank s-1
        chans = [NUM_BINS] + [_ref_bin(depths[b, n]) for n in order]
        g = np.empty((NSLOT, P, J), np.float32)
        xpix = xt[b].reshape(P, J, C)
        for s, ch in enumerate(chans):
            g[s] = xpix[:, :, ch]
        xg[b] = np.transpose(g, (1, 0, 2)).reshape(P, NSLOT * J).astype(
            ml_dtypes.bfloat16
        )

    return xsum, xg, smalls


def kernel(depth_logits, gt_boxes2d, gt_boxes3d, gt_center_depth, num_gt_per_img):
    depth_logits = np.asarray(depth_logits, dtype=np.float32)
    gt_boxes2d = np.asarray(gt_boxes2d, dtype=np.float32)
    gt_center_depth = np.asarray(gt_center_depth, dtype=np.float32)
    assert int(num_gt_per_img) == N

    xsum, xg, smalls = _host_prep(depth_logits, gt_boxes2d, gt_center_depth)

    if "nc" not in _CACHE:
        _CACHE["nc"] = _build()
    nc = _CACHE["nc"]

    in_maps = []
    for b in range(B):
        in_maps.append(
            {
                "xsum": np.ascontiguousarray(xsum[b]),
                "xg": np.ascontiguousarray(xg[b]),
                "smalls": np.ascontiguousarray(smalls[b]),
            }
        )

    res = run_bass_kernel_spmd(nc, in_maps, core_ids=list(range(B)))
    LAST_RESULT[0] = res
    total = 0.0
    for b in range(B):
        total += float(res.results[b]["outv"][0, 0])
    return np.float32(total / (B * H * W))
